# revision 37
# baseline (speedup 1.0000x reference)
"""Trainium2 Bass kernel for masked cross-attention (nn_Attention_21440476741938).

Reference computation (b=2, n=4096, n_txt=128, c=1536, c_ctx=4096, h=24, d=64):
    q = x @ Wq;  k = context @ Wk;  v = context @ Wv        (multi-head, d=64)
    out = softmax(q k^T / sqrt(d) + mask) v;  y = out @ Wo

Sharding across 8 NeuronCores: core i -> batch b=i//4, quarter j=i%4.
  Each core computes k/v projections for its 384 feature columns (6 heads)
  of its batch; one packed 4-core AllGather shares full K^T / V per batch.
  The core then runs attention + output projection for its 1024 query
  tokens (all 24 heads).  Outputs are disjoint -> no second collective.

The PE instruction stream is ordered to be gap-free (PE is the bottleneck
at ~190us of streaming at the power-throttled 13/16 clock):
  k/v proj (DMA-paced, fires the AllGather by ~55us) interleaved with
  qT proj qs=0  ->  qT proj qs=1  ->  attention chunks qt=0 (+ output
  proj qt=0 interleaved)  ->  qt=1 (+ output proj qt=1).
Inputs are host-relaid (packed kv quarters, per-output-chunk Wq/Wo
blocks, per-partition-contiguous x) so DMA stays ahead of the PE; a
dummy partition_broadcast early in the stream preloads the GpSimd
extended-isa library (~14us) off the critical path.
"""

import ml_dtypes
import numpy as np

import concourse.bass as bass
import concourse.bacc as bacc
import concourse.mybir as mybir
import concourse.tile as tile
from concourse.bass_utils import run_bass_kernel_spmd

F32 = mybir.dt.float32
BF16 = mybir.dt.bfloat16

B, NQ, NKV, CIN, CCTX, C = 2, 4096, 128, 1536, 4096, 1536
H, D = 24, 64
SCALE = float(D) ** -0.5
NCORES = 8
QTOK = NQ * B // NCORES          # 1024 query tokens per core
FSH = C // 4                     # 384 feature columns per core in phase 1
NCH = C // 128                   # 12 feature chunks
CCH = CCTX // 128                # 32 context-feature chunks
MASK_NEG = -60.0                 # exp(-60) ~ 8.8e-27: negligible vs valid terms


def build_nc():
    nc = bacc.Bacc("TRN2", target_bir_lowering=False, debug=False,
                   num_devices=NCORES)

    # xqin[qs][p][c][f]: rhs for qT projection, contiguous per partition
    xqin = nc.dram_tensor("xqin", [2, 128, NCH, 512], BF16, kind="ExternalInput").ap()
    # wqin[fc][p][c][col]: one contiguous [128,1536] block per output chunk
    wqin = nc.dram_tensor("wqin", [NCH, 128, NCH * 128], BF16, kind="ExternalInput").ap()
    woin = nc.dram_tensor("woin", [NCH, 128, NCH * 128], BF16, kind="ExternalInput").ap()
    # kvin[q] = 8 chunks of [ctx chunk (128 kv) | Wk shard | Wv shard]
    kvin = nc.dram_tensor("kvin", [4, 128, 8 * (NKV + 2 * FSH)], BF16,
                          kind="ExternalInput").ap()
    biasin = nc.dram_tensor("biasin", [NKV, 1], F32, kind="ExternalInput").ap()
    onesin = nc.dram_tensor("onesin", [128, 64], BF16, kind="ExternalInput").ap()
    eyein = nc.dram_tensor("eyein", [128, 128], BF16, kind="ExternalInput").ap()
    yT = nc.dram_tensor("yT", [C, QTOK], F32, kind="ExternalOutput").ap()

    with tile.TileContext(nc) as tc:
        _build_graph(nc, tc, xqin, wqin, woin, kvin, biasin, onesin,
                     eyein, yT)
    nc.compile()
    return nc


def _build_graph(nc, tc, xqin, wqin, woin, kvin, biasin, onesin, eyein, yT):
    Exp = mybir.ActivationFunctionType.Exp

    with (
        tc.tile_pool(name="dram", bufs=1, space="DRAM") as dram,
        tc.tile_pool(name="persist", bufs=1) as persist,
        tc.tile_pool(name="consts", bufs=1) as consts,
    ):
        # ---- constants (tiles up front; DMAs issued after the hot loads)
        ones_t = consts.tile([128, 64], BF16)
        ones_sb = ones_t[:, 0:1]       # (128,1) lhsT for column sums
        bias_sb = consts.tile([NKV, 1], F32)
        eye_sb = consts.tile([128, 128], BF16)

        # ---- DRAM bounce buffers for the packed AllGather
        kv_ag_in = dram.tile([2 * FSH, NKV], BF16)
        kv_full = dram.tile([8 * FSH, NKV], BF16)

        kT_sb = persist.tile([128, C], BF16)
        v_sb = persist.tile([128, C], BF16)
        qT_sb = persist.tile([128, NCH * QTOK], BF16)
        outT_sb = persist.tile([128, NCH * QTOK], BF16)

        with (
            tc.tile_pool(name="wqxq", bufs=1) as wqxq,
            tc.tile_pool(name="qtps", bufs=3, space="PSUM") as qtps,
            tc.tile_pool(name="p1sb", bufs=1) as p1sb,
            tc.tile_pool(name="p1kv", bufs=4) as p1kv,
            tc.tile_pool(name="p1psum", bufs=1, space="PSUM") as p1ps,
            tc.tile_pool(name="p1psum_t", bufs=2, space="PSUM") as p1ps_t,
        ):
            # qT-projection inputs, relaid on host for clean DMAs.
            xq_sb = wqxq.tile([128, 2 * NCH * 512], BF16)
            wq_sb = wqxq.tile([128, NCH * NCH * 128], BF16)
            KVW = NKV + 2 * FSH      # 896 cols per kv chunk
            kv_tiles = [p1kv.tile([128, 8 * KVW], BF16, name="kv_t")
                        for _ in range(4)]

            def load_wq_block(fc):
                nc.sync.dma_start(wq_sb[:, 1536 * fc:1536 * (fc + 1)], wqin[fc])

            # DMA priority order: qT starter inputs first, kv quarters
            # interleaved with the wq blocks so both streams stay ahead of
            # the PE and the AllGather can fire by ~45us.
            nc.sync.dma_start(xq_sb[:, 0:1536],
                              xqin[0].rearrange("p c f -> p (c f)")[:, 0:1536])
            load_wq_block(0)
            nc.sync.dma_start(xq_sb[:, 1536:6144],
                              xqin[0].rearrange("p c f -> p (c f)")[:, 1536:6144])
            for q in range(4):
                nc.sync.dma_start(kv_tiles[q][:], kvin[q])
                load_wq_block(1 + q)
            nc.scalar.dma_start(eye_sb[:], eyein)
            for fc in range(5, NCH):
                load_wq_block(fc)
            nc.sync.dma_start(xq_sb[:, 6144:12288],
                              xqin[1].rearrange("p c f -> p (c f)"))
            nc.scalar.dma_start(ones_t[:], onesin)
            nc.scalar.dma_start(bias_sb[:], biasin)

            # dummy broadcast: forces the GpSimd extended-isa library load
            # (~14us) to happen during the DMA-bound prefix instead of
            # stalling the first attention chunk; must precede the AllGather
            # completion wait in the GpSimd stream
            warm_bc = p1sb.tile([128, 1], F32)
            nc.gpsimd.partition_broadcast(warm_bc[:], bias_sb[0:1, 0:1])

            # ---------- k/v shard projection + AllGather (first in PE order
            # so the collective fires early; qT groups fill PE idle) --------
            k_ps = p1ps.tile([NKV, FSH], F32)
            v_ps = p1ps.tile([NKV, FSH], F32)
            for c in range(CCH):
                t = kv_tiles[c // 8]
                off = (c % 8) * KVW
                nc.tensor.matmul(k_ps[:], t[:, off:off + NKV],
                                 t[:, off + NKV:off + NKV + FSH],
                                 start=(c == 0), stop=(c == CCH - 1))
                nc.tensor.matmul(v_ps[:], t[:, off:off + NKV],
                                 t[:, off + NKV + FSH:off + KVW],
                                 start=(c == 0), stop=(c == CCH - 1))

            # v shard: natural layout -> second half of the packed AG input
            v_stage = p1sb.tile([NKV, FSH], BF16)
            nc.scalar.copy(v_stage[:], v_ps[:])
            v_dst = (kv_ag_in.rearrange("(x pk) k -> x (pk k)", x=2)[1:2, :]
                     .rearrange("o (p f) -> (o p) f", p=128))
            nc.scalar.dma_start(v_dst, v_stage[:])

            # k shard: transpose (128kv, 384f) -> (384f, 128kv), first half
            k_nat = p1sb.tile([NKV, FSH], BF16)
            nc.scalar.copy(k_nat[:], k_ps[:])
            kT_stage = p1sb.tile([128, 3 * NKV], BF16)
            for s in range(3):
                kt_ps = p1ps_t.tile([128, 128], BF16, name="kt_ps")
                nc.tensor.transpose(kt_ps[:], k_nat[:, 128 * s:128 * (s + 1)],
                                    eye_sb[:])
                nc.scalar.copy(kT_stage[:, 128 * s:128 * (s + 1)], kt_ps[:])
            nc.scalar.dma_start(
                kv_ag_in[0:FSH, :].rearrange("(s p) k -> p s k", p=128),
                kT_stage.rearrange("p (s k) -> p s k", s=3))

            groups = [[0, 1, 2, 3], [4, 5, 6, 7]]
            nc.gpsimd.collective_compute(
                "AllGather", mybir.AluOpType.bypass, replica_groups=groups,
                ins=[kv_ag_in[:].opt()], outs=[kv_full[:].opt()])

            # ---------- qT projection ----------
            def qt_group_direct(fc, qs):
                q_ps = qtps.tile([128, 512], F32, name="q_ps")
                for c in range(NCH):
                    nc.tensor.matmul(
                        q_ps[:],
                        wq_sb[:, 1536 * fc + 128 * c:1536 * fc + 128 * (c + 1)],
                        xq_sb[:, 6144 * qs + 512 * c:6144 * qs + 512 * (c + 1)],
                        start=(c == 0), stop=(c == NCH - 1))
                nc.vector.tensor_copy(
                    qT_sb[:, QTOK * fc + 512 * qs:QTOK * fc + 512 * (qs + 1)],
                    q_ps[:])

            for qs in range(2):
                for fc in range(NCH):
                    qt_group_direct(fc, qs)

        # ---------- unpack the AllGather result ----------
        # rank g's kT rows -> kT_sb blocks 3g..3g+2
        for g in range(4):
            nc.sync.dma_start(
                kT_sb[:, 384 * g:384 * (g + 1)].rearrange(
                    "p (s k) -> p s k", s=3),
                kv_full[768 * g:768 * g + 384, :].rearrange(
                    "(s p) k -> p s k", p=128))
            v_src = (kv_full
                     .rearrange("(gg x pk) k -> gg x (pk k)", gg=4, x=2)
                     [g:g + 1, 1:2, :]
                     .rearrange("go o (p f) -> (go o p) f", p=128))
            nc.sync.dma_start(v_sb[:, FSH * g:FSH * (g + 1)], v_src)

        # ================= attention + output projection =================
        with (
            tc.tile_pool(name="wop", bufs=1) as wop,
            tc.tile_pool(name="expp", bufs=6) as expp,
            tc.tile_pool(name="recipf", bufs=3) as recipf,
            tc.tile_pool(name="bcsb", bufs=3) as bcsb,
            tc.tile_pool(name="ytsb", bufs=3) as ytsb,
            tc.tile_pool(name="scps", bufs=2, space="PSUM") as scps,
            tc.tile_pool(name="denps", bufs=2, space="PSUM") as denps,
            tc.tile_pool(name="ovps", bufs=2, space="PSUM") as ovps,
            tc.tile_pool(name="ytps", bufs=2, space="PSUM") as ytps,
        ):
            # Wo resident in bf16, per-output-chunk blocks; the DMAs overlap
            # the attention pipeline lead-in (wq/xq SBUF is free by now)
            wo_sb = wop.tile([128, NCH * NCH * 128], BF16)
            for oc in range(NCH):
                nc.sync.dma_start(wo_sb[:, 1536 * oc:1536 * (oc + 1)], woin[oc])

            def emit_yt(oc, qt):
                y_ps = ytps.tile([128, 512], F32, name="y_ps")
                for c in range(NCH):
                    nc.tensor.matmul(
                        y_ps[:],
                        wo_sb[:, 1536 * oc + 128 * c:1536 * oc + 128 * (c + 1)],
                        outT_sb[:, QTOK * c + 512 * qt:QTOK * c + 512 * qt + 512],
                        start=(c == 0), stop=(c == NCH - 1))
                y_sb = ytsb.tile([128, 512], F32, name="y_sb")
                nc.scalar.copy(y_sb[:], y_ps[:])
                nc.sync.dma_start(
                    yT[128 * oc:128 * (oc + 1), 512 * qt:512 * qt + 512],
                    y_sb[:])

            _attention(nc, tc, qT_sb, kT_sb, v_sb, outT_sb, bias_sb,
                       ones_sb, expp, recipf, bcsb,
                       scps, denps, ovps, Exp, emit_yt)

            # remaining yT groups (qt=1)
            for oc in range(NCH):
                emit_yt(oc, 1)


def _attention(nc, tc, qT_sb, kT_sb, v_sb, outT_sb, bias_sb, ones_sb,
               expp, recipf, bcsb, scps, denps, ovps, Exp, emit_yt):
    """Attention over 24 (qt, c2) chunks, software-pipelined in 3 stages:
      A(i):   scores + exp               (PE, ACT)
      B(i-1): den colsums + attn.v + reciprocal + partition-broadcast
              (PE, DVE, GpSimd)
      C(i-2): normalize multiplies       (DVE)
    so no engine ever waits on a same-iteration cross-engine hop."""
    chunks = [(qt, c2) for qt in range(2) for c2 in range(NCH)]
    n = len(chunks)
    state = {}

    def stage_a(i):
        qt, c2 = chunks[i]
        exps = []
        for hh in range(2):
            sc_ps = scps.tile([NKV, 512], F32, name="sc_ps")
            nc.tensor.matmul(
                sc_ps[:],
                kT_sb[64 * hh:64 * hh + 64, 128 * c2:128 * (c2 + 1)],
                qT_sb[64 * hh:64 * hh + 64,
                      QTOK * c2 + 512 * qt:QTOK * c2 + 512 * qt + 512],
                start=True, stop=True)
            exp_sb = expp.tile([NKV, 512], BF16, name="exp_sb")
            nc.scalar.activation(exp_sb[:], sc_ps[:], Exp,
                                 bias=bias_sb[:], scale=SCALE)
            exps.append(exp_sb)
        state[i] = {"exps": exps}

    def stage_b(i):
        qt, c2 = chunks[i]
        st = state[i]
        exps = st["exps"]
        ov_ps = ovps.tile([128, 512], F32, name="ov_ps")
        recip_pair = recipf.tile([1, 1024], F32, name="recip_pair")
        dens = []
        for hh in range(2):
            den_ps = denps.tile([1, 512], F32, name="den_ps")
            nc.tensor.matmul(den_ps[:], ones_sb, exps[hh][:],
                             start=True, stop=True)
            dens.append(den_ps)
        for hh in range(2):
            h = 2 * c2 + hh
            nc.tensor.matmul(
                ov_ps[64 * hh:64 * hh + 64, :],
                v_sb[:, 64 * h:64 * h + 64],
                exps[hh][:], start=True, stop=True)
        for hh in range(2):
            nc.vector.reciprocal_approx_fast(
                recip_pair[0:1, 512 * hh:512 * hh + 512], dens[hh][:])
        bc_sb = bcsb.tile([128, 1024], F32, name="bc_sb")
        nc.gpsimd.partition_broadcast(bc_sb[:], recip_pair[:])
        st["ov"] = ov_ps
        st["bc"] = bc_sb

    def stage_c(i):
        qt, c2 = chunks[i]
        st = state.pop(i)
        ov_ps, bc_sb = st["ov"], st["bc"]
        ocol = QTOK * c2 + 512 * qt
        nc.vector.tensor_mul(outT_sb[0:64, ocol:ocol + 512],
                             ov_ps[0:64, :], bc_sb[0:64, 0:512])
        nc.vector.tensor_mul(outT_sb[64:128, ocol:ocol + 512],
                             ov_ps[64:128, :], bc_sb[64:128, 512:1024])

    for i in range(n + 2):
        if i < n:
            stage_a(i)
        if 1 <= i and i - 1 < n:
            stage_b(i - 1)
        if i - 2 >= 0:
            stage_c(i - 2)
            # once the qt=0 half is fully normalized (chunk 11 done), start
            # feeding its output-projection groups between attention chunks
            if emit_yt is not None and i - 2 >= NCH - 1 and i - 2 < 2 * NCH - 1:
                emit_yt(i - 2 - (NCH - 1), 0)


_NC_CACHE = None


def _get_nc():
    global _NC_CACHE
    if _NC_CACHE is None:
        _NC_CACHE = build_nc()
    return _NC_CACHE


def make_in_maps(x, context, context_mask, Wq, Wk, Wv, Wo):
    x = np.ascontiguousarray(np.asarray(x, dtype=np.float32))
    context = np.asarray(context, dtype=np.float32)
    context_mask = np.asarray(context_mask)
    Wq = np.ascontiguousarray(np.asarray(Wq, dtype=np.float32))
    Wk = np.asarray(Wk, dtype=np.float32)
    Wv = np.asarray(Wv, dtype=np.float32)
    Wo = np.ascontiguousarray(np.asarray(Wo, dtype=np.float32))

    bf = ml_dtypes.bfloat16
    eye = np.eye(128, dtype=bf)
    ones = np.ones((128, 64), dtype=bf)
    # [in=(c p)][out=(fc col)] -> [fc][p][c*128+col]
    wq_bf = np.ascontiguousarray(
        Wq.reshape(NCH, 128, NCH, 128).transpose(2, 1, 0, 3)
        .reshape(NCH, 128, NCH * 128).astype(bf))
    wo_bf = np.ascontiguousarray(
        Wo.reshape(NCH, 128, NCH, 128).transpose(2, 1, 0, 3)
        .reshape(NCH, 128, NCH * 128).astype(bf))
    ctx_by_b = [context[b].T.reshape(CCH, 128, NKV).astype(bf)
                for b in range(B)]
    in_maps = []
    for i in range(NCORES):
        b, j = i // 4, i % 4
        bias = np.where(context_mask[b], 0.0, MASK_NEG).astype(np.float32)[:, None]
        xTc = x[b, QTOK * j:QTOK * (j + 1), :].T          # (1536, 1024)
        # [in=(c p)][tok=(qs f)] -> [qs][p][c][f]
        xqin = np.ascontiguousarray(
            xTc.reshape(NCH, 128, 2, 512).transpose(2, 1, 0, 3).astype(bf))
        wkv = np.concatenate([Wk[:, FSH * j:FSH * (j + 1)],
                              Wv[:, FSH * j:FSH * (j + 1)]],
                             axis=1).astype(bf).reshape(CCH, 128, 2 * FSH)
        # [c][p][ctx|wk|wv] -> 4 quarters of 8 chunks, per-partition packed
        kvin = (np.concatenate([ctx_by_b[b], wkv], axis=2)
                .reshape(4, 8, 128, NKV + 2 * FSH).transpose(0, 2, 1, 3)
                .reshape(4, 128, 8 * (NKV + 2 * FSH)))
        kvin = np.ascontiguousarray(kvin)
        in_maps.append({
            "xqin": xqin,
            "wqin": wq_bf,
            "woin": wo_bf,
            "kvin": kvin,
            "biasin": bias,
            "onesin": ones,
            "eyein": eye,
        })
    return in_maps


def kernel(x, context, context_mask, Wq, Wk, Wv, Wo):
    in_maps = make_in_maps(x, context, context_mask, Wq, Wk, Wv, Wo)
    nc = _get_nc()
    res = run_bass_kernel_spmd(nc, in_maps, core_ids=list(range(NCORES)))

    y = np.empty((B, NQ, C), dtype=np.float32)
    for i in range(NCORES):
        b, j = i // 4, i % 4
        y[b, QTOK * j:QTOK * (j + 1), :] = res.results[i]["yT"].T
    return y
